# revision 8
# baseline (speedup 1.0000x reference)
"""Raw-Bacc v6: bf16 end-to-end, bias folded into PSUM drain, data DMAs
dispatched first, 4-chunk/4-bank pipeline.

out[n, c] = pf[c, n] + v0[c],  v0 = Wv @ age + bv

Math: every K row and V row of the cross-attention is identical (K/V come
from one broadcast age vector), so softmax weights are uniform and
attended == v0. The module collapses to a transpose plus a broadcast add.

Pipeline per 512-col chunk b (4 per core):
  in-DMA chunk b -> PE transposes 4 tiles into PSUM bank b -> DVE drains
  bank b with fused bias add (osb = pg + vbc, bf16) -> out-DMA b
  ([128 p, 4 t, 128 c] -> contiguous 256KB DRAM row-block).
vbc[p, c] = v0[c] comes from one PE matmul ageb^T @ wvt of a host-packed
wab [128, 256] (cols 0:128 = age column bcast + ones row + zero pad,
cols 128:256 = Wv^T + bv row + zero pad).
"""

import numpy as np

N_CORES = 8
B, C, D, H, W = 1, 128, 16, 32, 32
N = D * H * W
NSH = N // N_CORES       # 2048
AGE = 64
CHUNK = 512              # input dma chunk width == psum bank width
NCH = NSH // CHUNK       # 4
NT = NSH // 128          # 16 tiles


def build_nc():
    import concourse.bacc as bacc
    import concourse.mybir as mybir
    from contextlib import ExitStack

    f32 = mybir.dt.float32
    bf16 = mybir.dt.bfloat16
    nc = bacc.Bacc(
        "TRN2", target_bir_lowering=False, debug=False, num_devices=N_CORES)
    pf = nc.dram_tensor("pf", [C, NSH], bf16, kind="ExternalInput")
    wab = nc.dram_tensor("wab", [128, 256], f32, kind="ExternalInput")
    iden = nc.dram_tensor("iden", [128, 128], bf16, kind="ExternalInput")
    out = nc.dram_tensor("out", [NSH, C], bf16, kind="ExternalOutput")

    with ExitStack() as ctx:
        e = ctx.enter_context
        sid = e(nc.semaphore("sid"))
        swx = e(nc.semaphore("swx"))
        sin = [e(nc.semaphore(f"sin{k}")) for k in range(NCH)]
        sv = e(nc.semaphore("sv"))
        spe = e(nc.semaphore("spe"))
        sdv = e(nc.semaphore("sdv"))
        sout = e(nc.semaphore("sout"))

        identsb = e(nc.sbuf_tensor("identsb", [128, 128], bf16))
        wabsb = e(nc.sbuf_tensor("wabsb", [128, 256], f32))
        vbc = e(nc.sbuf_tensor("vbc", [128, 512], bf16))
        pft = e(nc.sbuf_tensor("pft", [C, NSH], bf16))
        osb = e(nc.sbuf_tensor("osb", [128, NSH], bf16))
        pgs = [e(nc.psum_tensor(f"pg{b}", [128, 512], bf16)) for b in range(4)]
        vps = e(nc.psum_tensor("vps", [128, 128], f32))
        block = e(nc.Block())

        def pg_tile(t):
            return pgs[t // 4][:, (t % 4) * 128:(t % 4 + 1) * 128]

        def out_dma(eng, b):
            eng.wait_ge(sdv, b + 1)
            eng.dma_start(
                out=out[b * 512:(b + 1) * 512, :].rearrange(
                    "(t p) c -> p t c", p=128),
                in_=osb[:, b * 512:(b + 1) * 512].rearrange(
                    "p (t c) -> p t c", c=128),
            ).then_inc(sout, 16)

        @block.sync
        def _(sync):
            sync.dma_start(
                out=pft[:, 0 * CHUNK:1 * CHUNK],
                in_=pf[:, 0 * CHUNK:1 * CHUNK]).then_inc(sin[0], 16)
            sync.dma_start(out=identsb[:], in_=iden[:]).then_inc(sid, 16)
            sync.dma_start(
                out=pft[:, 2 * CHUNK:3 * CHUNK],
                in_=pf[:, 2 * CHUNK:3 * CHUNK]).then_inc(sin[2], 16)
            out_dma(sync, 0)
            out_dma(sync, 2)
            sync.wait_ge(sout, 16 * NCH)

        @block.scalar
        def _(scalar):
            scalar.dma_start(out=wabsb[:], in_=wab[:]).then_inc(swx, 16)
            scalar.dma_start(
                out=pft[:, 1 * CHUNK:2 * CHUNK],
                in_=pf[:, 1 * CHUNK:2 * CHUNK]).then_inc(sin[1], 16)
            scalar.dma_start(
                out=pft[:, 3 * CHUNK:4 * CHUNK],
                in_=pf[:, 3 * CHUNK:4 * CHUNK]).then_inc(sin[3], 16)
            out_dma(scalar, 1)
            out_dma(scalar, 3)

        @block.tensor
        def _(tensor):
            tensor.wait_ge(swx, 16)
            tensor.matmul(
                vps[:], wabsb[:, 0:128], wabsb[:, 128:256],
            ).then_inc(sv, 1)
            tensor.wait_ge(sid, 16)
            for t in range(NT):
                if t % 4 == 0:
                    tensor.wait_ge(sin[t // 4], 16)
                tensor.transpose(
                    pg_tile(t),
                    pft[:, t * 128:(t + 1) * 128],
                    identsb[:],
                ).then_inc(spe, 1)

        @block.vector
        def _(vector):
            import concourse.mybir as mybir

            vector.wait_ge(sv, 1)
            for j in range(4):
                vector.tensor_copy(vbc[:, j * 128:(j + 1) * 128], vps[:])
            for b in range(4):
                vector.wait_ge(spe, 4 * (b + 1))
                vector.tensor_tensor(
                    osb[:, b * 512:(b + 1) * 512], pgs[b][:], vbc[:],
                    mybir.AluOpType.add,
                ).then_inc(sdv, 1)

    nc.finalize()
    return nc


_CACHE = {}
LAST_RESULTS = None


def kernel(**inputs):
    global LAST_RESULTS
    import ml_dtypes
    from concourse.bass_utils import run_bass_kernel_spmd

    bf16 = ml_dtypes.bfloat16
    if "nc" not in _CACHE:
        _CACHE["nc"] = build_nc()
    nc = _CACHE["nc"]

    pf_full = np.asarray(
        inputs["pixel_features"], dtype=np.float32).reshape(C, N).astype(bf16)
    age = np.asarray(inputs["age_features"], dtype=np.float32).reshape(AGE)
    Wv = np.asarray(inputs["Wv"], dtype=np.float32)
    bv = np.asarray(inputs["bv"], dtype=np.float32)
    wab_np = np.zeros((128, 256), dtype=np.float32)
    wab_np[0:AGE, 0:128] = age[:, None]
    wab_np[AGE, 0:128] = 1.0
    wab_np[0:AGE, 128:256] = Wv.T
    wab_np[AGE, 128:256] = bv
    iden_np = np.eye(128, dtype=bf16)

    in_maps = [
        {
            "pf": np.ascontiguousarray(pf_full[:, i * NSH:(i + 1) * NSH]),
            "wab": wab_np,
            "iden": iden_np,
        }
        for i in range(N_CORES)
    ]
    res = run_bass_kernel_spmd(nc, in_maps, core_ids=list(range(N_CORES)))
    LAST_RESULTS = res
    out = np.concatenate([res.results[i]["out"] for i in range(N_CORES)], axis=0)
    return out.astype(np.float32).reshape(B, N, C)
